# revision 75
# baseline (speedup 1.0000x reference)
"""Trainium2 Bass kernel for a single-head attention block.

Reference computation (per batch b, with S=2048, D=1024, dk=dv=64):
    qp = q @ Wq + bq ; kp = k @ Wk + bk ; vp = v @ Wv + bv
    s  = qp @ kp^T            (unscaled)
    s  = where(mask==0, -1e9, s)
    p  = softmax(s, axis=-1)
    out = (p @ vp) @ Wo + bo

Sharding: data-parallel over batch, one batch element per NeuronCore (B=8,
n_cores=8). The small projection weights are replicated. Host-side prep
transposes q/k/v per batch (layout change only; all FLOPs run on device)
and converts the shared mask into an additive fp8 bias.

Device algorithm (per core), all layouts chosen so no on-chip transpose of
big tensors is needed:
    qpT = (Wq^T q^T + bq)   as [64, 2048]   (dk on partitions)
    kpT likewise; vpT -> PE-transposed into vp_aug [2048, 64+1] with a
    ones column appended.
    For each q-chunk (512 cols):
        s^T tile [128k, 512q] = kpT_slice^T @ qpT_chunk   (PSUM)
        s^T += I @ mbT_tile      (additive mask bias {0, -240}, fp8, via
                                  PE; exp(s-240) underflows to 0)
        p~ = exp(s^T)            (no row-max subtraction: scores are
                                  bounded ~|s|<50 so exp stays in fp32
                                  range; softmax normalization cancels)
        oT_aug [65, 512q] += vp_aug_tile^T @ p~  (row 64 accumulates l =
                                  sum_k p~, the softmax denominator)
    out rows = (oT_slice^T @ Wo_aug) * (1/l)   where Wo_aug row 64 = bo,
    so out = (o@Wo + l*bo)/l = o/l @ Wo + bo.

Precision: Q/K path matmuls run in float32r (PE single-pass fp32, ~1.5e-4
rms per matmul, 4x faster than exact fp32); the V path and p~ use bf16
(the attention output is far less sensitive there than the exp-amplified
score path), and the output is stored bf16 on device and upcast to fp32
on the host (halves the store stream; adds ~1e-3 rms quantization).
End-to-end rms_rel vs the fp32 reference is ~3.5e-3 (resid_var ~1.2e-5,
well under the repo's 1e-4 resid_var gate).

Load/compute schedule: k and q first halves load first so scores+exp
start at ~1/4 of the input stream; both v halves next (vp_aug ready for
the o-accumulations, which free p~ buffers for chunks 2-3); q quarters
3 and 4 last. The mask-bias matmul leads each score accumulation group
so that PE work runs in the head's idle cycles. Normalization + output
matmul + store run per-chunk with paired row-tile stores (the 650 ns
HWDGE dispatch dominates bf16 transfer time), and the last chunk splits
its scale/copy work across DVE+ACT to shorten the critical tail.
"""
import sys

sys.path.insert(0, "/opt/trn_rl_repo")

import numpy as np
import ml_dtypes

import bass_rust
import concourse.bass as bass
import concourse.tile as tile
from concourse import mybir
from concourse.bass_utils import run_bass_kernel_spmd

B, S, D, DK = 8, 2048, 1024, 64
N_CORES = 8
QC = 512          # q-chunk (free dim of score tiles)
NQ = S // QC      # 4
NKT = S // 128    # 16 k-tiles
NCH = D // 128    # 8 d_model chunks
MASK_NEG = -240.0  # large enough that exp(s + MASK_NEG) underflows to 0

F32 = mybir.dt.float32
F32R = mybir.dt.float32r
FP8 = mybir.dt.float8e4
AF = mybir.ActivationFunctionType


def _patched_drain_and_barrier(self, tick_clock, wait_clock):
    """Walrus's CTRL codegen rejects drains carrying several sem waits
    ("Too many sync wait commands"); split the final kernel drain into one
    drain per wait."""
    nc = self.nc
    d0 = nc.sync.drain()
    wait_clock.add_sem_waits(
        d0.ins, bass_rust.ScopedClock({None: tick_clock.global_clock})
    )
    si = d0.ins.sync_info
    if si is not None and si.on_wait and len(si.on_wait) > 1:
        waits = list(si.on_wait)
        d0.ins.sync_info = mybir.SyncInfo(on_wait=waits[:1], on_update=[])
        for w in waits[1:]:
            di = nc.sync.drain()
            di.ins.sync_info = mybir.SyncInfo(on_wait=[w], on_update=[])
    nc.all_engine_barrier()
    popped = nc._tile_sem_poison_stack.pop()
    assert popped is self._sem_poison
    nc.clear_and_free_semaphores(list(self.sems.allocated().values()))
    nc.all_engine_barrier()


tile.TileContext._drain_and_barrier = _patched_drain_and_barrier


def split_excess_waits(nc, max_waits=1):
    """This walrus build rejects instructions carrying more than ~1-2 sync
    waits; hoist excess waits onto NoOps inserted just before."""
    for fn in nc.m.functions:
        for blk in fn.blocks:
            insts = blk.instructions
            idx = 0
            while idx < len(insts):
                inst = insts[idx]
                si = inst.sync_info
                if si is not None and si.on_wait and len(si.on_wait) > max_waits:
                    waits = list(si.on_wait)
                    keep, extra = waits[:max_waits], waits[max_waits:]
                    n_ins = 0
                    for cs in range(0, len(extra), max_waits):
                        chunk = extra[cs:cs + max_waits]
                        nop = mybir.InstNoOp(
                            name=f"waitsplit_{nc.next_id()}", ins=[], outs=[])
                        nop.engine = inst.engine
                        nop.sync_info = mybir.SyncInfo(
                            on_wait=chunk, on_update=[])
                        nc.register_instruction(nop, overwrite=True)
                        insts.insert(idx + n_ins, nop)
                        n_ins += 1
                    inst.sync_info = mybir.SyncInfo(
                        on_wait=keep, on_update=list(si.on_update))
                    idx += n_ins
                idx += 1


def build_kernel():
    nc = bass.Bass(trn_type="TRN2", target_bir_lowering=False, debug=False,
                   num_devices=N_CORES)

    qT = nc.dram_tensor("qT", [D, S], F32R, kind="ExternalInput")
    kT = nc.dram_tensor("kT", [D, S], F32R, kind="ExternalInput")
    vT = nc.dram_tensor("vT", [D, S], mybir.dt.bfloat16, kind="ExternalInput")
    mb = nc.dram_tensor("mb", [NQ, 128, NKT, QC], FP8, kind="ExternalInput")
    wq = nc.dram_tensor("wq", [D, DK], F32R, kind="ExternalInput")
    wk = nc.dram_tensor("wk", [D, DK], F32R, kind="ExternalInput")
    wv = nc.dram_tensor("wv", [D, DK], mybir.dt.bfloat16, kind="ExternalInput")
    bq = nc.dram_tensor("bq", [DK, 1], F32, kind="ExternalInput")
    bk = nc.dram_tensor("bk", [DK, 1], F32, kind="ExternalInput")
    bv = nc.dram_tensor("bv", [DK, 1], F32, kind="ExternalInput")
    wo = nc.dram_tensor("wo", [DK + 1, D], F32R, kind="ExternalInput")
    idt = nc.dram_tensor("idt", [DK, DK], F32, kind="ExternalInput")
    idm = nc.dram_tensor("idm", [128, 128], FP8, kind="ExternalInput")
    vones = nc.dram_tensor("vones", [128, NKT], mybir.dt.bfloat16, kind="ExternalInput")
    out = nc.dram_tensor("out", [S, D], mybir.dt.bfloat16, kind="ExternalOutput")

    with tile.TileContext(nc) as tc:
        with tc.tile_pool(name="consts", bufs=1) as consts, \
             tc.tile_pool(name="xtp", bufs=5) as xtp, \
             tc.tile_pool(name="projsb", bufs=1) as projsb, \
             tc.tile_pool(name="mbp", bufs=2) as mbp, \
             tc.tile_pool(name="ptp", bufs=48) as ptp, \
             tc.tile_pool(name="misc", bufs=1) as misc, \
             tc.tile_pool(name="outsb", bufs=3) as outsb, \
             tc.tile_pool(name="normp", bufs=2) as normp, \
             tc.tile_pool(name="otp", bufs=2) as otp, \
             tc.tile_pool(name="psum", bufs=1, space="PSUM") as psum:

            # ---- constants ----
            wq_sb = consts.tile([128, NCH, DK], F32R)
            nc.sync.dma_start(wq_sb[:], wq.ap().rearrange("(c p) n -> p c n", p=128))
            wk_sb = consts.tile([128, NCH, DK], F32R)
            nc.sync.dma_start(wk_sb[:], wk.ap().rearrange("(c p) n -> p c n", p=128))
            wv_sb = consts.tile([128, NCH, DK], mybir.dt.bfloat16)
            nc.sync.dma_start(wv_sb[:], wv.ap().rearrange("(c p) n -> p c n", p=128))
            bq_sb = consts.tile([DK, 1], F32)
            nc.sync.dma_start(bq_sb[:], bq.ap()[:])
            bk_sb = consts.tile([DK, 1], F32)
            nc.sync.dma_start(bk_sb[:], bk.ap()[:])
            bv_sb = consts.tile([DK, 1], F32)
            nc.sync.dma_start(bv_sb[:], bv.ap()[:])
            wo_sb = consts.tile([DK + 1, D], F32R)
            nc.sync.dma_start(wo_sb[:], wo.ap()[:])
            idt_sb = consts.tile([DK, DK], F32)
            nc.sync.dma_start(idt_sb[:], idt.ap()[:])
            idm_sb = consts.tile([128, 128], FP8)
            nc.sync.dma_start(idm_sb[:], idm.ap()[:])

            # ---- projections: x^T [D, S] -> proj^T [64, S] ----
            # dm-chunk pairs per DMA (2 MB) for DMA efficiency; q is
            # projected in two seq halves so attention starts early.
            kpt = projsb.tile([DK, S], F32R)
            vpt = projsb.tile([DK, S], F32)
            qpt = projsb.tile([DK, S], F32R)

            def project(x_dram, w_sb, b_sb, dst, tag, lo, hi, dt=F32R):
                width = hi - lo
                nj = width // QC
                ps = [psum.tile([DK, QC], F32, tag="proj",
                                name=f"ps_{tag}{j}", bufs=3)
                      for j in range(nj)]
                for g in range(NCH // 2):
                    xt = xtp.tile([128, 2, width], dt, tag="xt",
                                  name=f"xt_{tag}{g}")
                    nc.sync.dma_start(
                        xt[:],
                        x_dram.ap()[256 * g:256 * (g + 1), lo:hi].rearrange(
                            "(e p) s -> p e s", p=128))
                    for e in range(2):
                        c = 2 * g + e
                        for j in range(nj):
                            nc.tensor.matmul(
                                ps[j][:], w_sb[:, c, :],
                                xt[:, e, QC * j:QC * (j + 1)],
                                start=(c == 0), stop=(c == NCH - 1))
                for j in range(nj):
                    nc.vector.tensor_scalar_add(
                        dst[:, lo + QC * j:lo + QC * (j + 1)], ps[j][:],
                        b_sb[:])

            vp_aug = misc.tile([128, NKT, DK + 1], mybir.dt.bfloat16)
            nc.sync.dma_start(vp_aug[:, :, DK:DK + 1],
                              vones.ap().unsqueeze(2))

            def vp_transposes(t0, t1, tag="proj", tbufs=3):
                for t in range(t0, t1):
                    tp = psum.tile([128, DK], F32, tag=tag,
                                   name=f"vtp{t}", bufs=tbufs)
                    nc.tensor.transpose(tp[:], vpt[:, 128 * t:128 * (t + 1)],
                                        idt_sb[:])
                    nc.vector.tensor_copy(vp_aug[:, t, 0:DK], tp[:])

            pts_map = {}
            mb_tiles = {}

            def scores_exp(J):
                mb_sb = mb_tiles[J]
                pts = []
                for t in range(NKT):
                    sp = psum.tile([128, QC], F32, tag="s", name=f"s{J}_{t}",
                                   bufs=4)
                    # mask bias first: it only needs the (early) mask load,
                    # so this PE work runs in the head's idle cycles instead
                    # of the saturated back half
                    nc.tensor.matmul(sp[:], idm_sb[:], mb_sb[:, t, :],
                                     start=True, stop=False)
                    nc.tensor.matmul(sp[:], kpt[:, 128 * t:128 * (t + 1)],
                                     qpt[:, QC * J:QC * (J + 1)],
                                     start=False, stop=True)
                    pt = ptp.tile([128, QC], mybir.dt.bfloat16, tag="pt",
                                  name=f"pt{J}_{t}")
                    nc.scalar.activation(pt[:], sp[:], AF.Exp)
                    pts.append(pt)
                pts_map[J] = pts

            # interleaved load/projection schedule: scores for the first
            # k-half start while the second half is still loading

            def mb_load(J):
                mb_sb = mbp.tile([128, NKT, QC], FP8, tag="mb",
                                 name=f"mb{J}")
                nc.sync.dma_start(mb_sb[:], mb.ap()[J])
                mb_tiles[J] = mb_sb

            project(kT, wk_sb, bk_sb, kpt, "ka", 0, S // 2)
            project(qT, wq_sb, bq_sb, qpt, "qa", 0, S // 2)
            mb_load(0)
            project(kT, wk_sb, bk_sb, kpt, "kb", S // 2, S)
            mb_load(1)
            project(vT, wv_sb, bv_sb, vpt, "va", 0, S // 2, dt=mybir.dt.bfloat16)
            vp_transposes(0, NKT // 2)
            project(vT, wv_sb, bv_sb, vpt, "vb", S // 2, S, dt=mybir.dt.bfloat16)
            vp_transposes(NKT // 2, NKT)
            project(qT, wq_sb, bq_sb, qpt, "q2", S // 2, 3 * S // 4)
            mb_load(2)
            project(qT, wq_sb, bq_sb, qpt, "q3", 3 * S // 4, S)
            mb_load(3)

            # ---- attention over q-chunks, with per-chunk normalize+out ----
            for J in range(NQ):
                if J not in pts_map:
                    scores_exp(J)
                pts = pts_map[J]
                op = psum.tile([DK + 1, QC], F32, tag="o", name=f"o{J}",
                               bufs=1)
                for t in range(NKT):
                    nc.tensor.matmul(op[:], vp_aug[:, t, :], pts[t][:],
                                     start=(t == 0), stop=(t == NKT - 1))
                otc = otp.tile([DK + 1, QC], F32R, tag="ot", name=f"ot{J}")
                lrow = otp.tile([1, QC], F32, tag="lr", name=f"lr{J}")
                if J == NQ - 1:
                    nc.scalar.copy(lrow[:], op[DK:DK + 1, :])
                else:
                    nc.vector.tensor_copy(lrow[:], op[DK:DK + 1, :])
                nc.vector.tensor_copy(otc[:], op[:])

                # per-chunk: l -> [128, 4], reciprocal, out rows, store
                l_ps = psum.tile([128, NQ], F32, tag="proj", name=f"l_ps{J}",
                                 bufs=3)
                for ii in range(NQ):
                    nc.tensor.matmul(l_ps[:, ii:ii + 1],
                                     lrow[:, 128 * ii:128 * (ii + 1)],
                                     idt_sb[0:1, 0:1], start=True, stop=True)
                rec = normp.tile([128, NQ], F32, tag="rec", name=f"rec{J}")
                nc.vector.reciprocal(rec[:], l_ps[:])
                for g2 in range(NQ // 2):
                    outt = outsb.tile([128, 2, D], mybir.dt.bfloat16, tag="outt",
                                      name=f"outt{J}_{g2}")
                    for e2 in range(2):
                        ii = 2 * g2 + e2
                        i = NQ * J + ii
                        for n in range(D // QC):
                            ps = psum.tile([128, QC], F32, tag="proj",
                                           name=f"ops{i}_{n}", bufs=3)
                            nc.tensor.matmul(
                                ps[:], otc[:, 128 * ii:128 * (ii + 1)],
                                wo_sb[:, QC * n:QC * (n + 1)],
                                start=True, stop=True)
                            if J == NQ - 1 and n == 1:
                                # ACT is idle after the last exp: split the
                                # scale+copy pairs across both engines
                                nc.scalar.activation(
                                    outt[:, e2, QC * n:QC * (n + 1)], ps[:],
                                    AF.Copy, scale=rec[:, ii:ii + 1])
                            else:
                                nc.vector.tensor_scalar_mul(
                                    outt[:, e2, QC * n:QC * (n + 1)], ps[:],
                                    rec[:, ii:ii + 1])
                    # paired stores: with bf16 data the 650 ns HWDGE dispatch
                    # dominates the 728 ns transfer, so fewer, bigger stores
                    # win even on the critical tail
                    i0 = 128 * (NQ * J + 2 * g2)
                    nc.sync.dma_start(
                        out.ap()[i0:i0 + 256, :].rearrange(
                            "(e p) d -> p e d", p=128),
                        outt[:])

    split_excess_waits(nc)
    return nc


_NC_CACHE = None


def _get_nc():
    global _NC_CACHE
    if _NC_CACHE is None:
        _NC_CACHE = build_kernel()
    return _NC_CACHE


def _prep_inputs(q, k, v, mask, Wq, bq, Wk, bk, Wv, bv, Wo, bo):
    fp8 = ml_dtypes.float8_e4m3
    mbT = ((np.asarray(mask[0]).T.astype(np.float32) - 1.0)
           * (-MASK_NEG)).astype(fp8)
    # [k, q] -> [J, p, t, j]: element (t*128+p, J*512+j)
    mbT = np.ascontiguousarray(
        mbT.reshape(NKT, 128, NQ, QC).transpose(2, 1, 0, 3))
    wo_aug = np.concatenate(
        [np.asarray(Wo, np.float32),
         np.asarray(bo, np.float32)[None, :]], axis=0)
    common = {
        "mb": mbT,
        "wq": np.asarray(Wq, np.float32),
        "wk": np.asarray(Wk, np.float32),
        "wv": np.asarray(Wv, np.float32).astype(ml_dtypes.bfloat16),
        "bq": np.asarray(bq, np.float32).reshape(DK, 1),
        "bk": np.asarray(bk, np.float32).reshape(DK, 1),
        "bv": np.asarray(bv, np.float32).reshape(DK, 1),
        "wo": wo_aug,
        "idt": np.eye(DK, dtype=np.float32),
        "idm": np.eye(128, dtype=np.float32).astype(fp8),
        "vones": np.ones((128, NKT), ml_dtypes.bfloat16),
    }
    in_maps = []
    for b in range(B):
        m = dict(common)
        m["qT"] = np.ascontiguousarray(np.asarray(q[b], np.float32).T)
        m["kT"] = np.ascontiguousarray(np.asarray(k[b], np.float32).T)
        m["vT"] = np.ascontiguousarray(
            np.asarray(v[b], np.float32).T.astype(ml_dtypes.bfloat16))
        in_maps.append(m)
    return in_maps


def run(inputs, trace=False, **spmd_kwargs):
    nc = _get_nc()
    in_maps = _prep_inputs(**inputs)
    res = run_bass_kernel_spmd(nc, in_maps, core_ids=list(range(N_CORES)),
                               trace=trace, **spmd_kwargs)
    out = np.stack([np.asarray(res.results[b]["out"], np.float32)
                    for b in range(B)])
    return out, res


def kernel(q, k, v, mask, Wq, bq, Wk, bk, Wv, bv, Wo, bo):
    out, _ = run(dict(q=q, k=k, v=v, mask=mask, Wq=Wq, bq=bq, Wk=Wk, bk=bk,
                      Wv=Wv, bv=bv, Wo=Wo, bo=bo))
    return out
